# revision 2
# baseline (speedup 1.0000x reference)
"""BrightnessLoss Trainium2 kernel (raw Bass, 8-core data parallel), v2.

reference:
    V(x)   = max_c(clip(x, 0, 1))        over channel dim (RGB)
    result = mean(|V(pred) - V(target)|) over (N, H, W)

Identities used on device (bulk units, ACT-assisted path):
    clip(max(r,g,b),0,1) == max_c(clip(x,0,1))          (clip is monotone)
    u := relu(max3)  (free in the DVE stt: (m1 max 0) max B)
    W := Relu(1 - u) == 1 - clip(m, 0, 1)
    |Vp - Vt| == |Wp - Wt|
    sum|Wp - Wt| == 2*sum max(Wp,Wt) - sum Wp - sum Wt

Last unit (DVE-only path, no ACT round trip):
    vp := min(relu(max3_p), 1) == V(pred)   via stt (m max 0) max B ; (raw min 1) min raw
    sum|vp - vt| == sum max(vp,vt) - sum min(vp,vt)    (two accum stts)

Schedule (the stream is HBM-bound at ~358 GB/s/core; everything else hides):
  - 10 units; imgs 0-2 in 1024-wide halves, img 3 split (1024, 640, 256, 128).
  - Ring split: SP ring (sync) takes u0,u2,u4,u7,u8 (12.2 MB); ACT ring
    (scalar) takes u1,u3,u5,u6,u9 (13.0 MB) so the tiny last unit u9 lands
    last, alone, at full rate.
  - 4-deep input buffers; every input DMA issue is hoisted so each ring
    always has >=2 transfers queued (no mid-stream starvation).
  - ONE output DMA at the very end on the long-quiet SP ring; its
    completion receipt hides under the fixed end-of-NEFF semaphore sweep.
  - Bass.__init__'s all-engine barrier is suppressed so the first DMA
    issues ~0.5us earlier; a gpsimd guard sem protects the const-1.0
    bias tile that ACT's Relu reads.
Host combines partials in float64.
"""

import numpy as np

N_CORES = 8
N_IMG = 4  # 32 / 8
C = 3
P = 128
F = 2048  # 512*512 / 128
N_PIX = 32 * 512 * 512

# (img, col_offset, width, ring)  ring: 0 = SP/sync, 1 = ACT/scalar
UNITS = [
    (0, 0, 1024, 0),
    (0, 1024, 1024, 1),
    (1, 0, 1024, 0),
    (1, 1024, 1024, 1),
    (2, 0, 1024, 0),
    (2, 1024, 1024, 1),
    (3, 0, 1024, 1),
    (3, 1024, 640, 0),
    (3, 1664, 256, 0),
    (3, 1920, 128, 1),  # last: tiny, DVE-only, lands last on the ACT ring
]
N_UNITS = len(UNITS)
N_BULK = N_UNITS - 1  # units 0..8 use the ACT W path
SLOTS = 4
N_COLS = 3 * N_BULK + 2  # 27 bulk cols + (sum max, sum min) for the last unit


def _build_program():
    from contextlib import ExitStack

    import concourse.bass as bass
    import concourse.mybir as mybir

    fp32 = mybir.dt.float32
    Alu = mybir.AluOpType
    Act = mybir.ActivationFunctionType

    # Suppress the framework barrier at the end of Bass.__init__ (after the
    # const-AP memsets): engines then enter the body without a rendezvous and
    # the first input DMA issues earlier.  The only preamble state the body
    # reads is the const-1.0 bias tile (ACT Relu bias); a gpsimd-side guard
    # sem below re-establishes that one ordering edge.
    _cls_aeb = bass.Bass.all_engine_barrier
    bass.Bass.all_engine_barrier = lambda *a, **k: None
    try:
        # detect_race_conditions=False: the raw-mode CoreSim race detector
        # can't see same-engine program-order (DVE TT -> STT RAW); hardware
        # engines execute in order.
        nc = bass.Bass(
            "TRN2",
            target_bir_lowering=False,
            debug=False,
            detect_race_conditions=False,
        )
    finally:
        bass.Bass.all_engine_barrier = _cls_aeb

    pred = nc.dram_tensor("pred", [N_IMG, C, P, F], fp32, kind="ExternalInput").ap()
    targ = nc.dram_tensor("target", [N_IMG, C, P, F], fp32, kind="ExternalInput").ap()
    out = nc.dram_tensor("partials", [P, N_COLS], fp32, kind="ExternalOutput").ap()

    fc = 1024  # max unit width

    with ExitStack() as ctx:
        sb = lambda name, shape: ctx.enter_context(nc.sbuf_tensor(name, shape, fp32))
        sem = lambda name: ctx.enter_context(nc.semaphore(name))

        inb = [[sb(f"in{sl}{s}", [P, C * fc]) for s in range(2)] for sl in range(SLOTS)]
        ub = [[sb(f"u{sl}{s}", [P, fc]) for s in range(2)] for sl in range(SLOTS)]
        wb = [[sb(f"w{sl}{s}", [P, fc]) for s in range(2)] for sl in range(SLOTS)]
        m1 = sb("m1", [P, fc])
        scr = sb("scratch", [P, fc])
        rawp = sb("rawp", [P, 128])
        vp = sb("vp", [P, 128])
        acc = sb("acc", [P, N_COLS])
        guard_buf = sb("guard_buf", [P, 1])

        ip = [sem(f"ip{s}") for s in range(SLOTS)]  # pred DMA done, per slot
        it = [sem(f"it{s}") for s in range(SLOTS)]  # targ DMA done, per slot
        u_sem = sem("u")      # DVE stt per unit-side (2/unit)
        act_sem = sem("act")  # ACT W per unit-side (2/unit)
        gp_sem = sem("gp")    # DVE accums (9 bulk + 2 last = 11)
        out_sem = sem("outd")
        cready = sem("cready")  # gpsimd: const-AP memsets retired

        # per-unit slot bookkeeping
        slot_of = [u % SLOTS for u in range(N_UNITS)]
        use_of = [u // SLOTS for u in range(N_UNITS)]  # k-th user of its slot

        def dma_in(eng, side_idx, u):
            img, off, w, _ = UNITS[u]
            side = (pred, targ)[side_idx]
            s_sem = (ip, it)[side_idx][slot_of[u]]
            src = side[img, :, :, off : off + w].rearrange("c p f -> p c f")
            eng.dma_start(
                out=inb[slot_of[u]][side_idx][:, : C * w].rearrange(
                    "p (c f) -> p c f", c=C
                ),
                in_=src,
            ).then_inc(s_sem, 16)

        block = ctx.enter_context(nc.Block(no_gpsimd_drain=True))

        @block.gpsimd
        def _(gp):
            # Executes after the preamble const-AP memsets in gpsimd program
            # order; signals ACT that the const-1.0 bias tile is valid.
            gp.memset(guard_buf[:, :], 0.0).then_inc(cready, 1)

        @block.sync
        def _(sync):
            # SP ring: u0, u2, u4, u7, u8 then the single output DMA.
            for u in (0, 2, 4, 7, 8):
                if u >= SLOTS:
                    # WAR on inb[slot]: stts of the slot's previous unit
                    prev = u - SLOTS
                    sync.wait_ge(u_sem, 2 * prev + 1)
                dma_in(sync, 0, u)
                if u >= SLOTS:
                    sync.wait_ge(u_sem, 2 * (u - SLOTS) + 2)
                dma_in(sync, 1, u)
            sync.wait_ge(gp_sem, N_BULK + 2)
            # No out_sem wait after the write: the block-exit drain fences the
            # HWDGE ring before NEFF completion (receipt hides under the
            # end-of-NEFF semaphore sweep).
            sync.dma_start(out=out[:, :], in_=acc[:, :]).then_inc(out_sem, 16)

        @block.vector
        def _(vector):
            def accum(u):
                # max(Wp, Wt) elementwise, accum_out = per-partition sum
                w = UNITS[u][2]
                sl = slot_of[u]
                vector.wait_ge(act_sem, 2 * (u + 1))
                vector.scalar_tensor_tensor(
                    scr[:, :w],
                    wb[sl][0][:, :w],
                    0.0,
                    wb[sl][1][:, :w],
                    op0=Alu.bypass,
                    op1=Alu.max,
                    accum_out=acc[:, 3 * u : 3 * u + 1],
                ).then_inc(gp_sem, 1)

            for u in range(N_BULK):
                w = UNITS[u][2]
                sl, k = slot_of[u], use_of[u]
                t = inb[sl]
                for s in range(2):
                    vector.wait_ge((ip, it)[s][sl], 16 * (k + 1))
                    vector.tensor_max(m1[:, :w], t[s][:, 0:w], t[s][:, w : 2 * w])
                    if u >= SLOTS:
                        # WAR on ub[sl][s]: ACT's W of unit u-4 (its reader)
                        vector.wait_ge(act_sem, 2 * (u - SLOTS) + s + 1)
                    vector.scalar_tensor_tensor(
                        ub[sl][s][:, :w],
                        m1[:, :w],
                        0.0,
                        t[s][:, 2 * w : 3 * w],
                        op0=Alu.max,
                        op1=Alu.max,
                    ).then_inc(u_sem, 1)
                if u > 0:
                    accum(u - 1)
            accum(N_BULK - 1)

            # Last unit: DVE-only, w=128, slot computed generically.
            u = N_UNITS - 1
            w = UNITS[u][2]
            sl, k = slot_of[u], use_of[u]
            t = inb[sl]
            vector.wait_ge(ip[sl], 16 * (k + 1))
            vector.tensor_max(m1[:, :w], t[0][:, 0:w], t[0][:, w : 2 * w])
            # rawp = relu(max3(pred)) = (m1 max 0) max B
            vector.scalar_tensor_tensor(
                rawp[:, :w], m1[:, :w], 0.0, t[0][:, 2 * w : 3 * w],
                op0=Alu.max, op1=Alu.max,
            )
            # vp = clip(max3_p) = (rawp min 1) min rawp   (min is idempotent)
            vector.scalar_tensor_tensor(
                vp[:, :w], rawp[:, :w], 1.0, rawp[:, :w],
                op0=Alu.min, op1=Alu.min,
            )
            vector.wait_ge(it[sl], 16 * (k + 1))
            vector.tensor_max(m1[:, :w], t[1][:, 0:w], t[1][:, w : 2 * w])
            vector.scalar_tensor_tensor(
                rawp[:, :w], m1[:, :w], 0.0, t[1][:, 2 * w : 3 * w],
                op0=Alu.max, op1=Alu.max,
            )
            # sum max(vp, vt) and sum min(vp, vt); vt = (rawt min 1)
            vector.scalar_tensor_tensor(
                scr[:, :w], rawp[:, :w], 1.0, vp[:, :w],
                op0=Alu.min, op1=Alu.max,
                accum_out=acc[:, 3 * N_BULK : 3 * N_BULK + 1],
            ).then_inc(gp_sem, 1)
            vector.scalar_tensor_tensor(
                scr[:, :w], rawp[:, :w], 1.0, vp[:, :w],
                op0=Alu.min, op1=Alu.min,
                accum_out=acc[:, 3 * N_BULK + 1 : 3 * N_BULK + 2],
            ).then_inc(gp_sem, 1)

        @block.scalar
        def _(scalar):
            # ACT ring: u1, u3 up front; u5, u6, u9 hoisted between W's so the
            # ring always has >=2 transfers queued.  The W of unit n waits on
            # DVE's stts (u_sem >= 2n+1 / 2n+2), which also covers the WAR
            # conditions of every hoisted DMA below it.
            dma_in(scalar, 0, 1)
            dma_in(scalar, 1, 1)
            dma_in(scalar, 0, 3)
            dma_in(scalar, 1, 3)
            scalar.wait_ge(cready, 1)  # const-1.0 bias tile valid

            def W(n):
                w = UNITS[n][2]
                sl = slot_of[n]
                for s in range(2):
                    scalar.wait_ge(u_sem, 2 * n + s + 1)
                    if n >= SLOTS:
                        # WAR on wb[sl][s]: accum of unit n-4 (its reader)
                        scalar.wait_ge(gp_sem, n - SLOTS + 1)
                    scalar.activation(
                        wb[sl][s][:, :w],
                        ub[sl][s][:, :w],
                        Act.Relu,
                        bias=1.0,
                        scale=-1.0,
                        accum_out=acc[:, 3 * n + 1 + s : 3 * n + 2 + s],
                    ).then_inc(act_sem, 1)

            W(0)
            W(1)  # waited u_sem >= 4: covers u5's WAR (stts of u1)
            dma_in(scalar, 0, 5)
            dma_in(scalar, 1, 5)
            W(2)  # waited u_sem >= 6: covers u6's WAR (stts of u2)
            dma_in(scalar, 0, 6)
            dma_in(scalar, 1, 6)
            W(3)
            W(4)
            W(5)  # waited u_sem >= 12: covers u9's WAR (stts of u5)
            dma_in(scalar, 0, 9)
            dma_in(scalar, 1, 9)
            W(6)
            W(7)
            W(8)

        # Skip the Block-exit all-engine barrier (~4.3us): every cross-engine
        # dependency is semaphore-gated and the per-engine exit drains
        # (no_gpsimd_drain path) still fence the DMA rings, so engines may
        # halt independently — NEFF completion waits for all engines anyway.
        nc.all_engine_barrier = lambda *a, **k: None

    del nc.all_engine_barrier  # restore class method
    return nc


_program = None


def _get_program():
    global _program
    if _program is None:
        _program = _build_program()
    return _program


def _finish(partials_list):
    """partials_list: per-core [P, N_COLS] f32.
    Bulk unit u cols [3u, 3u+1, 3u+2] = [sum max(Wp,Wt), sum Wp, sum Wt]:
      sum|Vp-Vt| over the unit = 2*col0 - col1 - col2.
    Last unit cols [27, 28] = [sum max(vp,vt), sum min(vp,vt)]:
      sum|Vp-Vt| = col27 - col28."""
    total = np.float64(0.0)
    for p in partials_list:
        p = p.astype(np.float64)
        b = p[:, : 3 * N_BULK]
        total += 2.0 * b[:, 0::3].sum() - b[:, 1::3].sum() - b[:, 2::3].sum()
        total += p[:, 3 * N_BULK].sum() - p[:, 3 * N_BULK + 1].sum()
    return np.array(total / N_PIX, dtype=np.float32)


def kernel(pred: np.ndarray, target: np.ndarray) -> np.ndarray:
    from concourse.bass_utils import run_bass_kernel_spmd

    nc = _get_program()
    pred = np.ascontiguousarray(pred, dtype=np.float32).reshape(
        N_CORES, N_IMG, C, P, F
    )
    target = np.ascontiguousarray(target, dtype=np.float32).reshape(
        N_CORES, N_IMG, C, P, F
    )
    in_maps = [{"pred": pred[i], "target": target[i]} for i in range(N_CORES)]
    res = run_bass_kernel_spmd(nc, in_maps, list(range(N_CORES)))
    return _finish([r["partials"] for r in res.results])


# revision 3
# speedup vs baseline: 1.0838x; 1.0838x over previous
"""BrightnessLoss Trainium2 kernel (raw Bass, 8-core data parallel), v3.

reference:
    V(x)   = max_c(clip(x, 0, 1))        over channel dim (RGB)
    result = mean(|V(pred) - V(target)|) over (N, H, W)

Identities (bulk units, ACT-assisted path):
    u := relu(max3)  (free in the DVE stt: (m1 max 0) max B)
    W := Relu(1 - u) == 1 - clip(max3, 0, 1)
    |Vp - Vt| == |Wp - Wt|;  sum|Wp-Wt| == 2*sum max(Wp,Wt) - sum Wp - sum Wt
Last unit (DVE-only, no ACT round trip):
    vp := min(relu(max3_p), 1) == V(pred)
    sum|vp - vt| == sum max(vp,vt) - sum min(vp,vt)

Schedule notes (HBM-bound at ~358 GB/s/core; keep the two HWDGE rings
alternating — continuous dual-streaming costs ~10% HBM rate, and queueing
more than ~2 large DMAs ahead stalls the sequencer on ring credits):
  - 10 units: imgs 0-2 in 1024 halves; img 3 split (1024, 640, 256, 128).
  - Rings: SP (sync) u0,u2,u4,u6,u9; ACT (scalar) u1,u3,u5,u7,u8.  SP
    carries more bytes so the tiny DVE-only u9 lands last, alone.
  - 3-deep input buffers: each ring's next DMA is issued one W earlier
    than the 2-slot scheme allowed, so neither ring ever runs dry.
  - ONE output DMA at the very end, from the by-then-quiet ACT ring; its
    receipt hides under the fixed end-of-NEFF semaphore sweep.
  - Bass.__init__'s all-engine barrier is suppressed (earlier first DMA);
    a gpsimd guard sem orders the const-1.0 bias tile for ACT's Relu.
Host combines partials in float64.
"""

import numpy as np

N_CORES = 8
N_IMG = 4  # 32 / 8
C = 3
P = 128
F = 2048  # 512*512 / 128
N_PIX = 32 * 512 * 512

# (img, col_offset, width, ring)  ring: 0 = SP/sync, 1 = ACT/scalar
UNITS = [
    (0, 0, 1024, 0),
    (0, 1024, 1024, 1),
    (1, 0, 1024, 0),
    (1, 1024, 1024, 1),
    (2, 0, 1024, 0),
    (2, 1024, 1024, 1),
    (3, 0, 1024, 0),
    (3, 1024, 640, 1),
    (3, 1664, 256, 1),
    (3, 1920, 128, 0),  # last: tiny, DVE-only, lands last on the SP ring
]
N_UNITS = len(UNITS)
N_BULK = N_UNITS - 1  # units 0..8 use the ACT W path
SLOTS = 3
N_COLS = 3 * N_BULK + 2  # 27 bulk cols + (sum max, sum min) for u9


def _build_program():
    from contextlib import ExitStack

    import concourse.bass as bass
    import concourse.mybir as mybir

    fp32 = mybir.dt.float32
    Alu = mybir.AluOpType
    Act = mybir.ActivationFunctionType

    # Suppress the framework barrier at the end of Bass.__init__ (after the
    # const-AP memsets): engines enter the body without a rendezvous and the
    # first input DMA issues earlier.  The only preamble state the body reads
    # is the const-1.0 bias tile (ACT Relu bias); the gpsimd guard sem below
    # re-establishes that one ordering edge.
    _cls_aeb = bass.Bass.all_engine_barrier
    bass.Bass.all_engine_barrier = lambda *a, **k: None
    try:
        # detect_race_conditions=False: the raw-mode CoreSim race detector
        # can't see same-engine program-order (DVE TT -> STT RAW); hardware
        # engines execute in order.
        nc = bass.Bass(
            "TRN2",
            target_bir_lowering=False,
            debug=False,
            detect_race_conditions=False,
        )
    finally:
        bass.Bass.all_engine_barrier = _cls_aeb

    pred = nc.dram_tensor("pred", [N_IMG, C, P, F], fp32, kind="ExternalInput").ap()
    targ = nc.dram_tensor("target", [N_IMG, C, P, F], fp32, kind="ExternalInput").ap()
    out = nc.dram_tensor("partials", [P, N_COLS], fp32, kind="ExternalOutput").ap()

    fc = 1024  # max unit width

    with ExitStack() as ctx:
        sb = lambda name, shape: ctx.enter_context(nc.sbuf_tensor(name, shape, fp32))
        sem = lambda name: ctx.enter_context(nc.semaphore(name))

        inb = [[sb(f"in{sl}{s}", [P, C * fc]) for s in range(2)] for sl in range(SLOTS)]
        ub = [[sb(f"u{sl}{s}", [P, fc]) for s in range(2)] for sl in range(2)]
        wb = [[sb(f"w{sl}{s}", [P, fc]) for s in range(2)] for sl in range(2)]
        m1 = sb("m1", [P, fc])
        scr = sb("scratch", [P, fc])
        rawp = sb("rawp", [P, 128])
        vp = sb("vp", [P, 128])
        acc = sb("acc", [P, N_COLS])
        guard_buf = sb("guard_buf", [P, 1])

        ip = [sem(f"ip{s}") for s in range(SLOTS)]  # pred DMA done, per slot
        it = [sem(f"it{s}") for s in range(SLOTS)]  # targ DMA done, per slot
        u_sem = sem("u")      # DVE stt per unit-side (2/unit)
        act_sem = sem("act")  # ACT W per unit-side (2/unit)
        gp_sem = sem("gp")    # DVE accums (9 bulk + 2 last = 11)
        out_sem = sem("outd")
        cready = sem("cready")  # gpsimd: const-AP memsets retired

        slot_of = [u % SLOTS for u in range(N_UNITS)]
        use_of = [u // SLOTS for u in range(N_UNITS)]

        def dma_in(eng, side_idx, u):
            img, off, w, _ = UNITS[u]
            side = (pred, targ)[side_idx]
            s_sem = (ip, it)[side_idx][slot_of[u]]
            src = side[img, :, :, off : off + w].rearrange("c p f -> p c f")
            eng.dma_start(
                out=inb[slot_of[u]][side_idx][:, : C * w].rearrange(
                    "p (c f) -> p c f", c=C
                ),
                in_=src,
            ).then_inc(s_sem, 16)

        block = ctx.enter_context(nc.Block(no_gpsimd_drain=True))

        @block.gpsimd
        def _(gp):
            # Runs after the preamble const-AP memsets in gpsimd program
            # order; signals ACT that the const-1.0 bias tile is valid.
            gp.memset(guard_buf[:, :], 0.0).then_inc(cready, 1)

        @block.sync
        def _(sync):
            # SP ring: u0, u2 up front (2 queued max), then WAR-paced.
            # WAR on inb[u%3]: stts of unit u-3 (its last reader).
            for u in (0, 2, 4, 6, 9):
                if u >= SLOTS:
                    sync.wait_ge(u_sem, 2 * (u - SLOTS) + 1)
                dma_in(sync, 0, u)
                if u >= SLOTS:
                    sync.wait_ge(u_sem, 2 * (u - SLOTS) + 2)
                dma_in(sync, 1, u)

        @block.vector
        def _(vector):
            def accum(u):
                # max(Wp, Wt) elementwise, accum_out = per-partition sum
                w = UNITS[u][2]
                vector.wait_ge(act_sem, 2 * (u + 1))
                vector.scalar_tensor_tensor(
                    scr[:, :w],
                    wb[u % 2][0][:, :w],
                    0.0,
                    wb[u % 2][1][:, :w],
                    op0=Alu.bypass,
                    op1=Alu.max,
                    accum_out=acc[:, 3 * u : 3 * u + 1],
                ).then_inc(gp_sem, 1)

            for u in range(N_BULK):
                w = UNITS[u][2]
                sl, k = slot_of[u], use_of[u]
                t = inb[sl]
                for s in range(2):
                    vector.wait_ge((ip, it)[s][sl], 16 * (k + 1))
                    vector.tensor_max(m1[:, :w], t[s][:, 0:w], t[s][:, w : 2 * w])
                    if u >= 2:
                        # WAR on ub[u%2][s]: ACT's W of unit u-2 (its reader)
                        vector.wait_ge(act_sem, 2 * (u - 1))
                    vector.scalar_tensor_tensor(
                        ub[u % 2][s][:, :w],
                        m1[:, :w],
                        0.0,
                        t[s][:, 2 * w : 3 * w],
                        op0=Alu.max,
                        op1=Alu.max,
                    ).then_inc(u_sem, 1)
                if u > 0:
                    accum(u - 1)
            accum(N_BULK - 1)

            # u9: DVE-only, w=128, lands last.
            u = N_UNITS - 1
            w = UNITS[u][2]
            sl, k = slot_of[u], use_of[u]
            t = inb[sl]
            vector.wait_ge(ip[sl], 16 * (k + 1))
            vector.tensor_max(m1[:, :w], t[0][:, 0:w], t[0][:, w : 2 * w])
            # rawp = relu(max3(pred)) = (m1 max 0) max B
            vector.scalar_tensor_tensor(
                rawp[:, :w], m1[:, :w], 0.0, t[0][:, 2 * w : 3 * w],
                op0=Alu.max, op1=Alu.max,
            )
            # vp = clip(max3_p, 0, 1) = (rawp min 1) min rawp  (min idempotent)
            vector.scalar_tensor_tensor(
                vp[:, :w], rawp[:, :w], 1.0, rawp[:, :w],
                op0=Alu.min, op1=Alu.min,
            )
            vector.wait_ge(it[sl], 16 * (k + 1))
            vector.tensor_max(m1[:, :w], t[1][:, 0:w], t[1][:, w : 2 * w])
            vector.scalar_tensor_tensor(
                rawp[:, :w], m1[:, :w], 0.0, t[1][:, 2 * w : 3 * w],
                op0=Alu.max, op1=Alu.max,
            )
            # sum max(vp,vt), sum min(vp,vt);  vt = (rawt min 1)
            vector.scalar_tensor_tensor(
                scr[:, :w], rawp[:, :w], 1.0, vp[:, :w],
                op0=Alu.min, op1=Alu.max,
                accum_out=acc[:, 3 * N_BULK : 3 * N_BULK + 1],
            ).then_inc(gp_sem, 1)
            vector.scalar_tensor_tensor(
                scr[:, :w], rawp[:, :w], 1.0, vp[:, :w],
                op0=Alu.min, op1=Alu.min,
                accum_out=acc[:, 3 * N_BULK + 1 : 3 * N_BULK + 2],
            ).then_inc(gp_sem, 1)

        @block.scalar
        def _(scalar):
            # ACT ring: u1 up front; each later unit is hoisted right after
            # the W whose u_sem wait covers its inb WAR (stts of unit u-3):
            # u3 after W_0 (>=1,2), u5 after W_2 (>=5,6), u7 after W_4
            # (>=9,10), u8 after W_5 (>=11,12).
            dma_in(scalar, 0, 1)
            dma_in(scalar, 1, 1)
            scalar.wait_ge(cready, 1)  # const-1.0 bias tile valid

            def W(n):
                w = UNITS[n][2]
                for s in range(2):
                    scalar.wait_ge(u_sem, 2 * n + s + 1)
                    if n >= 2:
                        # WAR on wb[n%2][s]: accum of unit n-2 (its reader)
                        scalar.wait_ge(gp_sem, n - 1)
                    scalar.activation(
                        wb[n % 2][s][:, :w],
                        ub[n % 2][s][:, :w],
                        Act.Relu,
                        bias=1.0,
                        scale=-1.0,
                        accum_out=acc[:, 3 * n + 1 + s : 3 * n + 2 + s],
                    ).then_inc(act_sem, 1)

            W(0)
            dma_in(scalar, 0, 3)
            dma_in(scalar, 1, 3)
            W(1)
            W(2)
            dma_in(scalar, 0, 5)
            dma_in(scalar, 1, 5)
            W(3)
            W(4)
            dma_in(scalar, 0, 7)
            dma_in(scalar, 1, 7)
            W(5)
            dma_in(scalar, 0, 8)
            dma_in(scalar, 1, 8)
            W(6)
            W(7)
            W(8)
            scalar.wait_ge(gp_sem, N_BULK + 2)
            # Single output DMA at the very end on the quiet ACT ring.  No
            # out_sem wait after it: the block-exit drain fences the ring and
            # the receipt hides under the end-of-NEFF semaphore sweep.
            scalar.dma_start(out=out[:, :], in_=acc[:, :]).then_inc(out_sem, 16)

        # Skip the Block-exit all-engine barrier: every cross-engine
        # dependency is semaphore-gated and the per-engine exit drains
        # (no_gpsimd_drain path) still fence the DMA rings, so engines may
        # halt independently — NEFF completion waits for all engines anyway.
        nc.all_engine_barrier = lambda *a, **k: None

    del nc.all_engine_barrier  # restore class method
    return nc


_program = None


def _get_program():
    global _program
    if _program is None:
        _program = _build_program()
    return _program


def _finish(partials_list):
    """partials_list: per-core [P, N_COLS] f32.
    Bulk unit u cols [3u, 3u+1, 3u+2] = [sum max(Wp,Wt), sum Wp, sum Wt]:
      sum|Vp-Vt| over the unit = 2*col0 - col1 - col2.
    Last unit cols [27, 28] = [sum max(vp,vt), sum min(vp,vt)]:
      sum|Vp-Vt| = col27 - col28."""
    total = np.float64(0.0)
    for p in partials_list:
        p = p.astype(np.float64)
        b = p[:, : 3 * N_BULK]
        total += 2.0 * b[:, 0::3].sum() - b[:, 1::3].sum() - b[:, 2::3].sum()
        total += p[:, 3 * N_BULK].sum() - p[:, 3 * N_BULK + 1].sum()
    return np.array(total / N_PIX, dtype=np.float32)


def kernel(pred: np.ndarray, target: np.ndarray) -> np.ndarray:
    from concourse.bass_utils import run_bass_kernel_spmd

    nc = _get_program()
    pred = np.ascontiguousarray(pred, dtype=np.float32).reshape(
        N_CORES, N_IMG, C, P, F
    )
    target = np.ascontiguousarray(target, dtype=np.float32).reshape(
        N_CORES, N_IMG, C, P, F
    )
    in_maps = [{"pred": pred[i], "target": target[i]} for i in range(N_CORES)]
    res = run_bass_kernel_spmd(nc, in_maps, list(range(N_CORES)))
    return _finish([r["partials"] for r in res.results])
